# revision 23
# baseline (speedup 1.0000x reference)
"""Trainium2 Bass kernel for nn_AutoregressivePrior (8-slot LSTM prior).

Strategy: pure data-parallel over batch (16384 rows -> 2048 per NeuronCore),
weights replicated. Feature-major dataflow on chip: every activation lives as
[feature_partition, batch_free] so LSTM matmul chains never transpose.

Mixed-precision matmuls tuned to the PE roofline (fp8 DoubleRow = 2x bf16):
  - W_hh @ h (70%% of FLOPs): fp8 e4m3 DoubleRow, W scaled 2^8, h scaled 2^4.
  - W_ih @ x for the i/f/o gates: fp8 DoubleRow (same scales).
  - W_ih @ x for the g gate (tanh, error-sensitive): bf16, W pre-scaled 2^12
    so its PSUM matches the fp8 product scale; one 1/4096 on the way out.
  - softplus head: fp8 DoubleRow reusing the gates' h8 tiles; mu head: bf16
    (mu is a direct output, fp8 there costs ~4%% rel err).
PSUM accumulates fp32; gate nonlinearities fused with bias+descale on ScalarE
straight out of PSUM; cell state held in fp16 in SBUF.

Per core the 2048-row batch is processed in two sequential passes of 1024
columns so all state fits in SBUF.

Inputs arrive as full-size numpy arrays; outputs are returned full-size
(zs, mus, sigmas) each [num_slots, 16384, 256] fp32, matching the reference.
"""

import sys

if "/opt/trn_rl_repo" not in sys.path:
    sys.path.insert(0, "/opt/trn_rl_repo")

import numpy as np
import ml_dtypes

BF16 = ml_dtypes.bfloat16
F8E4 = ml_dtypes.float8_e4m3

B = 16384
N_CORES = 8
B_LOC = B // N_CORES  # 2048
SCENE = 256
FEAT = 256
HID = 1024
G4 = 4 * HID  # 4096
NS = 8
P = 128
PO = FEAT // P  # 2
KH = HID // P  # 8
KX = FEAT // P  # 2
MT = G4 // P  # 32

PASS_W = 1024  # batch columns per pass on chip

SW = 256.0  # fp8 weight scale
SA = 16.0  # fp8 activation scale
INV_G = 1.0 / (SW * SA)  # gate PSUM descale (both fp8 and scaled-bf16 paths)
SSP = 128.0  # fp8 softplus-head weight scale
INV_SP = 1.0 / (SSP * SA)

_PATCHED = False


def _patch_tile_drain():
    """walrus in this toolchain rejects >1 sync-wait on a single instruction;
    split excess waits onto standalone single-wait EventSemaphore instructions
    that run on the same engine immediately before the original instruction."""
    global _PATCHED
    if _PATCHED:
        return
    import bass_rust
    import concourse.tile as tile
    from concourse import mybir
    from concourse.vector_clock import ScopedClock

    MAXW = 1
    _orig_lower = tile.TileContext._lower_ordered_insts

    def _lower_split_waits(self, ordered):
        nc = self.nc
        for bbn, insts in ordered.items():
            out = []
            for inst in insts:
                si = getattr(inst, "sync_info", None)
                if si is not None:
                    waits = list(si.on_wait)
                    if len(waits) > MAXW:
                        imm = [w for w in waits if w.wait_mode == "sem-ge-imm"]
                        other = [w for w in waits if w.wait_mode != "sem-ge-imm"]
                        assert len(other) <= MAXW, (inst.name, waits)
                        keep_n = MAXW - len(other)
                        if keep_n > 0:
                            move = imm[: len(imm) - keep_n]
                            keep = imm[len(imm) - keep_n :]
                        else:
                            move = imm
                            keep = []
                        for wt in move:
                            wi = mybir.InstEventSemaphore(
                                name=nc.get_next_instruction_name(),
                                ins=[],
                                outs=[],
                                engine=inst.engine,
                            )
                            wi.sync_info = bass_rust.SyncInfo(
                                on_wait=[wt], on_update=[]
                            )
                            out.append(wi)
                        si.on_wait = other + keep
                out.append(inst)
            insts[:] = out
        return _orig_lower(self, ordered)

    tile.TileContext._lower_ordered_insts = _lower_split_waits

    def _drain_and_barrier(self, tick_clock, wait_clock):
        nc = self.nc
        drain_inst = nc.sync.drain()
        wait_clock.add_sem_waits(
            drain_inst.ins, ScopedClock({None: tick_clock.global_clock})
        )
        si = drain_inst.ins.sync_info
        if si is not None and len(si.on_wait) > 1:
            waits = list(si.on_wait)
            si.on_wait = waits[:1]
            name2handle = {h.name: h for h in self.sems.allocated().values()}
            for w in waits[1:]:
                assert w.wait_mode == "sem-ge-imm", w
                nc.sync.wait_ge(name2handle[w.ant_name], w.wait_value)
        nc.all_engine_barrier()
        popped = nc._tile_sem_poison_stack.pop()
        assert popped is self._sem_poison
        nc.clear_and_free_semaphores(list(self.sems.allocated().values()))
        nc.all_engine_barrier()

    tile.TileContext._drain_and_barrier = _drain_and_barrier
    _PATCHED = True


def build(b_loc=B_LOC, w=PASS_W, n_slots=NS, mm_n=512):
    _patch_tile_drain()
    import concourse.bass as bass
    import concourse.tile as tile
    from concourse import mybir

    F32 = mybir.dt.float32
    BF = mybir.dt.bfloat16
    F16 = mybir.dt.float16
    F8 = mybir.dt.float8e4
    AF = mybir.ActivationFunctionType
    DR = mybir.MatmulPerfMode.DoubleRow
    MUL = mybir.AluOpType.mult

    n_pass = b_loc // w
    assert n_pass * w == b_loc
    chunks = [(c, min(mm_n, w - c)) for c in range(0, w, mm_n)]

    nc = bass.Bass()
    s_ext = nc.dram_tensor("s", [SCENE, b_loc], BF, kind="ExternalInput")
    eps_ext = nc.dram_tensor("eps", [NS, FEAT, b_loc], BF, kind="ExternalInput")
    wx8_ext = nc.dram_tensor("wx8", [FEAT, G4], F8, kind="ExternalInput")
    wxg_ext = nc.dram_tensor("wxg", [FEAT, HID], BF, kind="ExternalInput")
    wh8_ext = nc.dram_tensor("wh8", [HID, G4], F8, kind="ExternalInput")
    wmu_ext = nc.dram_tensor("wmu", [HID, FEAT], BF, kind="ExternalInput")
    wsp8_ext = nc.dram_tensor("wsp8", [HID, FEAT], F8, kind="ExternalInput")
    we_ext = nc.dram_tensor("we", [SCENE, FEAT], BF, kind="ExternalInput")
    bias_ext = nc.dram_tensor("bias", [P, 38], F32, kind="ExternalInput")
    # outputs in bf16 (host upcasts): halves store DMA; the fed-back z is
    # identical to the bf16 x_next the LSTM consumed anyway
    oz_ext = nc.dram_tensor("oz", [NS, FEAT, b_loc], BF, kind="ExternalOutput")
    omu_ext = nc.dram_tensor("omu", [NS, FEAT, b_loc], BF, kind="ExternalOutput")
    osg_ext = nc.dram_tensor("osg", [NS, FEAT, b_loc], BF, kind="ExternalOutput")

    with tile.TileContext(nc) as tc:
        with (
            tc.tile_pool(name="wp", bufs=1) as wp,
            tc.tile_pool(name="work", bufs=2) as dp,
            tc.tile_pool(name="psum", bufs=4, space="PSUM") as pp,
        ):
            # DMA order matters: the encoder and slot 0 need only we/s/wx/bias,
            # so load those before the big (4MB) W_hh to let PE start early.
            # s for BOTH passes is loaded upfront: 8KB/partition buys zero
            # mid-kernel dependency at the pass boundary.
            we_sb = wp.tile([P, KX, FEAT], BF, tag="we", name="we_sb")
            nc.sync.dma_start(we_sb[:], we_ext.rearrange("(ko p) m -> p ko m", p=P))
            bias_sb = wp.tile([P, 38], F32, tag="bias", name="bias_sb")
            nc.sync.dma_start(bias_sb[:], bias_ext[:])
            s_sb = wp.tile([P, KX, b_loc], BF, tag="s", name="s_sb")
            s_re = s_ext.rearrange("(po p) b -> p po b", p=P)
            # split per pass so pass-0's encoder starts after 1MB, not 2MB
            nc.sync.dma_start(s_sb[:, :, 0:w], s_re[:, :, 0:w])
            wx8_sb = wp.tile([P, KX, G4], F8, tag="wx8", name="wx8_sb")
            nc.sync.dma_start(wx8_sb[:], wx8_ext.rearrange("(ko p) m -> p ko m", p=P))
            wxg_sb = wp.tile([P, KX, HID], BF, tag="wxg", name="wxg_sb")
            nc.sync.dma_start(wxg_sb[:], wxg_ext.rearrange("(ko p) m -> p ko m", p=P))
            nc.sync.dma_start(s_sb[:, :, w:b_loc], s_re[:, :, w:b_loc])
            wmu_sb = wp.tile([P, KH, FEAT], BF, tag="wmu", name="wmu_sb")
            nc.sync.dma_start(wmu_sb[:], wmu_ext.rearrange("(ko p) m -> p ko m", p=P))
            wsp8_sb = wp.tile([P, KH, FEAT], F8, tag="wsp8", name="wsp8_sb")
            nc.sync.dma_start(
                wsp8_sb[:], wsp8_ext.rearrange("(ko p) m -> p ko m", p=P)
            )
            wh8_sb = wp.tile([P, KH, G4], F8, tag="wh8", name="wh8_sb")
            nc.sync.dma_start(wh8_sb[:], wh8_ext.rearrange("(ko p) m -> p ko m", p=P))

            # x/x8 as manually alternated persistent buffers: pool-tag cycling
            # across the encoder/slot scopes makes tile_validation fall back
            # to a conservative min-join that stalls the PE sequencer on the
            # full DVE drain at pass boundaries.
            xb = [
                wp.tile([P, KX, w], BF, tag=f"x{i}", name=f"x{i}_sb")
                for i in range(2)
            ]
            x8b = [
                wp.tile([P, KX, w], F8, tag=f"x8{i}", name=f"x8{i}_sb")
                for i in range(2)
            ]
            nx = 0

            for p_i in range(n_pass):
                col0 = p_i * w
                c_sb = dp.tile([P, KH, w], F16, tag="c", bufs=1, name="c_sb")

                # encoder: x0 = gelu(We @ S_loc.T + be), feature-major
                x_cur = xb[nx]
                for fi in range(PO):
                    ps = pp.tile([P, w], F32, tag="ps", name="ps_enc")
                    for k in range(KX):
                        for ci, (c0, cw) in enumerate(chunks):
                            nc.tensor.matmul(
                                ps[:, c0 : c0 + cw],
                                we_sb[:, k, fi * P : (fi + 1) * P],
                                s_sb[:, k, col0 + c0 : col0 + c0 + cw],
                                start=(k == 0),
                                stop=(k == KX - 1),
                            )
                    nc.scalar.activation(
                        x_cur[:, fi],
                        ps[:],
                        AF.Gelu,
                        bias=bias_sb[:, 32 + fi : 33 + fi],
                    )
                # fp8 copy of x0 for the i/f/o gate matmuls
                x8_cur = x8b[nx]
                nx = 1 - nx
                for fi in range(PO):
                    nc.vector.tensor_scalar_mul(x8_cur[:, fi], x_cur[:, fi], SA)

                h_prev = None
                h8_prev = None
                for t in range(n_slots):
                    eps_sb = dp.tile([P, PO, w], BF, tag="eps", bufs=2, name="eps_sb")
                    nc.sync.dma_start(
                        eps_sb[:],
                        eps_ext[t].rearrange("(po p) b -> p po b", p=P)[
                            :, :, col0 : col0 + w
                        ],
                    )
                    h_new = dp.tile([P, KH, w], BF, tag="h", bufs=2, name="h_sb")
                    h8_new = dp.tile([P, KH, w], F8, tag="h8", bufs=2, name="h8_sb")
                    # (r_idx, o_gate_tile) whose tanh(c)/h-multiply is
                    # deferred until after the NEXT r-block's gate evictions,
                    # so ScalarE's eviction stream never stalls on the DVE
                    # c-update (that stall starves PSUM recycling and PE).
                    pend = None

                    def flush_pend():
                        nonlocal pend
                        if pend is None:
                            return
                        rp, gop = pend
                        th = dp.tile([P, w], BF, tag="th", bufs=2, name="th_sb")
                        nc.scalar.activation(th[:], c_sb[:, rp], AF.Tanh)
                        nc.vector.tensor_mul(h_new[:, rp], gop[:], th[:])
                        # h8 = (o*16)*tanh(c) in one DVE op (fp8 out)
                        nc.vector.scalar_tensor_tensor(
                            h8_new[:, rp], gop[:], SA, th[:], MUL, MUL
                        )
                        pend = None

                    def emit_h_part(ps, m):
                        for kk in range(KH // 2):
                            for ci, (c0, cw) in enumerate(chunks):
                                nc.tensor.matmul(
                                    ps[:, c0 : c0 + cw],
                                    wh8_sb[:, 2 * kk : 2 * kk + 2, m * P : (m + 1) * P],
                                    h8_prev[:, 2 * kk : 2 * kk + 2, c0 : c0 + cw],
                                    start=(kk == 0),
                                    stop=False,
                                    perf_mode=DR,
                                )

                    def emit_x_part(ps, g, r, m, first):
                        if g == 2:
                            # tanh gate: bf16 x-path (W pre-scaled 4096)
                            for k in range(KX):
                                for ci, (c0, cw) in enumerate(chunks):
                                    nc.tensor.matmul(
                                        ps[:, c0 : c0 + cw],
                                        wxg_sb[:, k, r * P : (r + 1) * P],
                                        x_cur[:, k, c0 : c0 + cw],
                                        start=(first and k == 0),
                                        stop=(k == KX - 1),
                                    )
                        else:
                            for ci, (c0, cw) in enumerate(chunks):
                                nc.tensor.matmul(
                                    ps[:, c0 : c0 + cw],
                                    wx8_sb[:, 0:2, m * P : (m + 1) * P],
                                    x8_cur[:, 0:2, c0 : c0 + cw],
                                    start=first,
                                    stop=True,
                                    perf_mode=DR,
                                )

                    for r in range(KH):
                        gts = {}
                        # slot 0: c=0, so the forget gate is never used
                        gate_ids = (0, 2, 3) if h_prev is None else (0, 1, 2, 3)
                        pss = {}
                        if h_prev is not None and r == 0:
                            # first r-block of a slot: issue ALL h-side chains
                            # before any x-part, so the x8/x feedback chain
                            # from the previous slot's heads hides under ~7us
                            # of h-matmul work.
                            for g in gate_ids:
                                pss[g] = pp.tile([P, w], F32, tag="ps", name="ps_g")
                                emit_h_part(pss[g], g * KH + r)
                        for g in gate_ids:
                            m = g * KH + r
                            if g in pss:
                                ps = pss[g]
                                emit_x_part(ps, g, r, m, False)
                            else:
                                ps = pp.tile([P, w], F32, tag="ps", name="ps_g")
                                first = True
                                if h_prev is not None:
                                    emit_h_part(ps, m)
                                    first = False
                                emit_x_part(ps, g, r, m, first)
                            # o-gate is double-buffered: its consumer (the
                            # deferred h-multiply) runs one r-block late
                            gt = dp.tile(
                                [P, w],
                                BF,
                                tag=f"g{g}",
                                bufs=2 if g == 3 else 1,
                                name=f"g{g}_sb",
                            )
                            func = AF.Tanh if g == 2 else AF.Sigmoid
                            nc.scalar.activation(
                                gt[:],
                                ps[:],
                                func,
                                bias=bias_sb[:, m : m + 1],
                                scale=INV_G,
                            )
                            gts[g] = gt
                            # flush the previous r-block's deferred tanh/h
                            # chain right after the FIRST eviction: its th
                            # lands one eviction deep in the ScalarE queue
                            # instead of four, shortening the h8 straggler
                            # chain at slot boundaries.
                            if g == gate_ids[0]:
                                flush_pend()
                        gi, gf, gg, go = (gts.get(g) for g in range(4))
                        if h_prev is not None:
                            t1 = dp.tile([P, w], BF, tag="t1", bufs=1, name="t1_sb")
                            nc.vector.tensor_mul(t1[:], gi[:], gg[:])
                            t2 = dp.tile([P, w], F32, tag="t2", bufs=2, name="t2_sb")
                            nc.vector.tensor_mul(t2[:], gf[:], c_sb[:, r])
                            nc.vector.tensor_add(c_sb[:, r], t1[:], t2[:])
                        else:
                            nc.vector.tensor_mul(c_sb[:, r], gi[:], gg[:])
                        pend = (r, go)
                    flush_pend()

                    # heads: mu = Wmu.T @ h (bf16); softplus_pre = Wsp.T @ h8
                    # (fp8 DR). The sigma half runs first: its ACT chain
                    # (Exp -> Ln -> ze) is the long pole toward the next
                    # slot's x, and the Ln/ze ops overlap the mu matmuls.
                    mu_sb = dp.tile([P, PO, w], BF, tag="mu", bufs=1, name="mu_sb")
                    sg_sb = dp.tile([P, PO, w], BF, tag="sg", bufs=1, name="sg_sb")
                    x_next = xb[nx]
                    x8_next = x8b[nx]
                    nx = 1 - nx
                    last_slot = p_i == n_pass - 1 and t == n_slots - 1
                    # per po half: sp kk0..2, mu k0..5, sp kk3, mu k6..7 —
                    # the r6/r7 straggler chain (last gate evict -> tanh ->
                    # h/h8) hides under the r0..r5-only matmuls, and sigma's
                    # ACT chain (Exp -> Ln -> ze) still overlaps mu work.
                    for po in range(PO):
                        ps_sp = pp.tile([P, w], F32, tag="ps", name="ps_h")
                        for kk in range(3):
                            for ci, (c0, cw) in enumerate(chunks):
                                nc.tensor.matmul(
                                    ps_sp[:, c0 : c0 + cw],
                                    wsp8_sb[
                                        :, 2 * kk : 2 * kk + 2, po * P : (po + 1) * P
                                    ],
                                    h8_new[:, 2 * kk : 2 * kk + 2, c0 : c0 + cw],
                                    start=(kk == 0),
                                    stop=False,
                                    perf_mode=DR,
                                )
                        ps_mu = pp.tile([P, w], F32, tag="ps", name="ps_h")
                        for k in range(6):
                            for ci, (c0, cw) in enumerate(chunks):
                                nc.tensor.matmul(
                                    ps_mu[:, c0 : c0 + cw],
                                    wmu_sb[:, k, po * P : (po + 1) * P],
                                    h_new[:, k, c0 : c0 + cw],
                                    start=(k == 0),
                                    stop=False,
                                )
                        for ci, (c0, cw) in enumerate(chunks):
                            nc.tensor.matmul(
                                ps_sp[:, c0 : c0 + cw],
                                wsp8_sb[:, 6:8, po * P : (po + 1) * P],
                                h8_new[:, 6:8, c0 : c0 + cw],
                                start=False,
                                stop=True,
                                perf_mode=DR,
                            )
                        # softplus(u) = ln(1 + exp(u)); this toolchain has
                        # no softplus ACT table, but exp and ln share one.
                        nc.scalar.activation(
                            sg_sb[:, po],
                            ps_sp[:],
                            AF.Exp,
                            bias=bias_sb[:, 36 + po : 37 + po],
                            scale=INV_SP,
                        )
                        nc.scalar.activation(
                            sg_sb[:, po], sg_sb[:, po], AF.Ln, bias=1.0
                        )
                        zt = dp.tile([P, w], F32, tag="t2", bufs=2, name="ze_sb")
                        nc.vector.tensor_mul(zt[:], sg_sb[:, po], eps_sb[:, po])
                        for k in (6, 7):
                            for ci, (c0, cw) in enumerate(chunks):
                                nc.tensor.matmul(
                                    ps_mu[:, c0 : c0 + cw],
                                    wmu_sb[:, k, po * P : (po + 1) * P],
                                    h_new[:, k, c0 : c0 + cw],
                                    start=False,
                                    stop=(k == KH - 1),
                                )
                        nc.sync.dma_start(
                            osg_ext[t].rearrange("(po p) b -> p po b", p=P)[
                                :, po : po + 1, col0 : col0 + w
                            ],
                            sg_sb[:, po : po + 1],
                        )
                        nc.scalar.activation(
                            mu_sb[:, po],
                            ps_mu[:],
                            AF.Identity,
                            bias=bias_sb[:, 34 + po : 35 + po],
                        )
                        # z = mu + sigma*eps, written straight into the bf16
                        # x_next tile: it IS both the next slot's input and
                        # the z output
                        nc.vector.tensor_add(x_next[:, po], zt[:], mu_sb[:, po])
                        if not last_slot:
                            nc.vector.tensor_scalar_mul(
                                x8_next[:, po], x_next[:, po], SA
                            )
                        nc.sync.dma_start(
                            oz_ext[t].rearrange("(po p) b -> p po b", p=P)[
                                :, po : po + 1, col0 : col0 + w
                            ],
                            x_next[:, po : po + 1],
                        )
                        nc.sync.dma_start(
                            omu_ext[t].rearrange("(po p) b -> p po b", p=P)[
                                :, po : po + 1, col0 : col0 + w
                            ],
                            mu_sb[:, po : po + 1],
                        )
                    x_cur = x_next
                    x8_cur = x8_next
                    h_prev = h_new
                    h8_prev = h8_new
    return nc


_NC_CACHE = {}


def _get_nc(b_loc, w, n_slots):
    key = (b_loc, w, n_slots)
    if key not in _NC_CACHE:
        _NC_CACHE[key] = build(b_loc, w, n_slots)
    return _NC_CACHE[key]


def _prep_shared(We, be, W_ih, W_hh, b_ih, b_hh, W_mu, b_mu, W_sp, b_sp):
    f32 = np.float32
    wxT = np.ascontiguousarray(np.asarray(W_ih, f32).T)  # [FEAT, G4]
    wx8 = (wxT * SW).astype(F8E4)
    wxg = np.ascontiguousarray(wxT[:, 2 * HID : 3 * HID] * (SW * SA)).astype(BF16)
    wh8 = (np.ascontiguousarray(np.asarray(W_hh, f32).T) * SW).astype(F8E4)
    wmu = np.ascontiguousarray(np.asarray(W_mu, f32).T).astype(BF16)
    wsp8 = (np.ascontiguousarray(np.asarray(W_sp, f32).T) * SSP).astype(F8E4)
    we = np.ascontiguousarray(np.asarray(We, f32).T).astype(BF16)
    bg = (np.asarray(b_ih, f32) + np.asarray(b_hh, f32)).reshape(MT, P).T
    beT = np.asarray(be, f32).reshape(PO, P).T
    bmuT = np.asarray(b_mu, f32).reshape(PO, P).T
    bspT = np.asarray(b_sp, f32).reshape(PO, P).T
    bias = np.ascontiguousarray(
        np.concatenate([bg, beT, bmuT, bspT], axis=1), dtype=f32
    )
    return {
        "wx8": wx8,
        "wxg": wxg,
        "wh8": wh8,
        "wmu": wmu,
        "wsp8": wsp8,
        "we": we,
        "bias": bias,
    }


def _prep_in_maps(S, eps, shared, n_cores=N_CORES, b_loc=B_LOC):
    f32 = np.float32
    S = np.asarray(S, f32)
    eps = np.asarray(eps, f32)
    in_maps = []
    for ci in range(n_cores):
        rows = slice(ci * b_loc, (ci + 1) * b_loc)
        s_t = np.ascontiguousarray(S[rows].T).astype(BF16)
        eps_t = np.ascontiguousarray(eps[:NS, rows, :].transpose(0, 2, 1)).astype(
            BF16
        )
        in_maps.append({"s": s_t, "eps": eps_t, **shared})
    return in_maps


def _run(inputs, trace=False):
    from concourse.bass_utils import run_bass_kernel_spmd

    num_slots = int(inputs.get("num_slots", NS))
    nc = _get_nc(B_LOC, PASS_W, NS)
    shared = _prep_shared(
        inputs["We"], inputs["be"], inputs["W_ih"], inputs["W_hh"],
        inputs["b_ih"], inputs["b_hh"], inputs["W_mu"], inputs["b_mu"],
        inputs["W_sp"], inputs["b_sp"],
    )
    in_maps = _prep_in_maps(inputs["S"], inputs["eps"], shared)
    res = run_bass_kernel_spmd(
        nc, in_maps, core_ids=list(range(N_CORES)), trace=trace
    )
    zs = np.empty((NS, B, FEAT), np.float32)
    mus = np.empty((NS, B, FEAT), np.float32)
    sgs = np.empty((NS, B, FEAT), np.float32)
    for ci in range(N_CORES):
        rows = slice(ci * B_LOC, (ci + 1) * B_LOC)
        zs[:, rows, :] = res.results[ci]["oz"].transpose(0, 2, 1)
        mus[:, rows, :] = res.results[ci]["omu"].transpose(0, 2, 1)
        sgs[:, rows, :] = res.results[ci]["osg"].transpose(0, 2, 1)
    return (zs[:num_slots], mus[:num_slots], sgs[:num_slots]), res.exec_time_ns


def kernel(**inputs):
    out, _ = _run(inputs, trace=False)
    return out


# revision 28
# speedup vs baseline: 1.0046x; 1.0046x over previous
"""Trainium2 Bass kernel for nn_AutoregressivePrior (8-slot LSTM prior).

Strategy: pure data-parallel over batch (16384 rows -> 2048 per NeuronCore),
weights replicated. Feature-major dataflow on chip: every activation lives as
[feature_partition, batch_free] so LSTM matmul chains never transpose.

Mixed-precision matmuls tuned to the PE roofline (fp8 DoubleRow = 2x bf16):
  - W_hh @ h (70%% of FLOPs): fp8 e4m3 DoubleRow, W scaled 2^8, h scaled 2^4.
  - W_ih @ x for the i/f/o gates: fp8 DoubleRow (same scales).
  - W_ih @ x for the g gate (tanh, error-sensitive): bf16, W pre-scaled 2^12
    so its PSUM matches the fp8 product scale; one 1/4096 on the way out.
  - softplus head: fp8 DoubleRow reusing the gates' h8 tiles; mu head: bf16
    (mu is a direct output, fp8 there costs ~4%% rel err).
PSUM accumulates fp32; gate nonlinearities fused with bias+descale on ScalarE
straight out of PSUM; cell state held in fp16 in SBUF.

Per core the 2048-row batch is processed in two sequential passes of 1024
columns so all state fits in SBUF.

Inputs arrive as full-size numpy arrays; outputs are returned full-size
(zs, mus, sigmas) each [num_slots, 16384, 256] fp32, matching the reference.
"""

import sys

if "/opt/trn_rl_repo" not in sys.path:
    sys.path.insert(0, "/opt/trn_rl_repo")

import numpy as np
import ml_dtypes

BF16 = ml_dtypes.bfloat16
F8E4 = ml_dtypes.float8_e4m3

B = 16384
N_CORES = 8
B_LOC = B // N_CORES  # 2048
SCENE = 256
FEAT = 256
HID = 1024
G4 = 4 * HID  # 4096
NS = 8
P = 128
PO = FEAT // P  # 2
KH = HID // P  # 8
KX = FEAT // P  # 2
MT = G4 // P  # 32

PASS_W = 1024  # batch columns per pass on chip

SW = 256.0  # fp8 weight scale
SA = 16.0  # fp8 activation scale
INV_G = 1.0 / (SW * SA)  # gate PSUM descale (both fp8 and scaled-bf16 paths)
SSP = 128.0  # fp8 softplus-head weight scale
INV_SP = 1.0 / (SSP * SA)

_PATCHED = False


def _patch_tile_drain():
    """walrus in this toolchain rejects >1 sync-wait on a single instruction;
    split excess waits onto standalone single-wait EventSemaphore instructions
    that run on the same engine immediately before the original instruction."""
    global _PATCHED
    if _PATCHED:
        return
    import bass_rust
    import concourse.tile as tile
    from concourse import mybir
    from concourse.vector_clock import ScopedClock

    MAXW = 1
    _orig_lower = tile.TileContext._lower_ordered_insts

    def _lower_split_waits(self, ordered):
        nc = self.nc
        for bbn, insts in ordered.items():
            out = []
            for inst in insts:
                si = getattr(inst, "sync_info", None)
                if si is not None:
                    waits = list(si.on_wait)
                    if len(waits) > MAXW:
                        imm = [w for w in waits if w.wait_mode == "sem-ge-imm"]
                        other = [w for w in waits if w.wait_mode != "sem-ge-imm"]
                        assert len(other) <= MAXW, (inst.name, waits)
                        keep_n = MAXW - len(other)
                        if keep_n > 0:
                            move = imm[: len(imm) - keep_n]
                            keep = imm[len(imm) - keep_n :]
                        else:
                            move = imm
                            keep = []
                        for wt in move:
                            wi = mybir.InstEventSemaphore(
                                name=nc.get_next_instruction_name(),
                                ins=[],
                                outs=[],
                                engine=inst.engine,
                            )
                            wi.sync_info = bass_rust.SyncInfo(
                                on_wait=[wt], on_update=[]
                            )
                            out.append(wi)
                        si.on_wait = other + keep
                out.append(inst)
            insts[:] = out
        return _orig_lower(self, ordered)

    tile.TileContext._lower_ordered_insts = _lower_split_waits

    def _drain_and_barrier(self, tick_clock, wait_clock):
        nc = self.nc
        drain_inst = nc.sync.drain()
        wait_clock.add_sem_waits(
            drain_inst.ins, ScopedClock({None: tick_clock.global_clock})
        )
        si = drain_inst.ins.sync_info
        if si is not None and len(si.on_wait) > 1:
            waits = list(si.on_wait)
            si.on_wait = waits[:1]
            name2handle = {h.name: h for h in self.sems.allocated().values()}
            for w in waits[1:]:
                assert w.wait_mode == "sem-ge-imm", w
                nc.sync.wait_ge(name2handle[w.ant_name], w.wait_value)
        nc.all_engine_barrier()
        popped = nc._tile_sem_poison_stack.pop()
        assert popped is self._sem_poison
        nc.clear_and_free_semaphores(list(self.sems.allocated().values()))
        nc.all_engine_barrier()

    tile.TileContext._drain_and_barrier = _drain_and_barrier
    _PATCHED = True


def build(b_loc=B_LOC, w=PASS_W, n_slots=NS, mm_n=512):
    _patch_tile_drain()
    import concourse.bass as bass
    import concourse.tile as tile
    from concourse import mybir

    F32 = mybir.dt.float32
    BF = mybir.dt.bfloat16
    F16 = mybir.dt.float16
    F8 = mybir.dt.float8e4
    AF = mybir.ActivationFunctionType
    DR = mybir.MatmulPerfMode.DoubleRow
    MUL = mybir.AluOpType.mult

    n_pass = b_loc // w
    assert n_pass * w == b_loc
    chunks = [(c, min(mm_n, w - c)) for c in range(0, w, mm_n)]

    nc = bass.Bass()
    s_ext = nc.dram_tensor("s", [SCENE, b_loc], BF, kind="ExternalInput")
    eps_ext = nc.dram_tensor("eps", [NS, FEAT, b_loc], BF, kind="ExternalInput")
    wx8_ext = nc.dram_tensor("wx8", [FEAT, G4], F8, kind="ExternalInput")
    wxg_ext = nc.dram_tensor("wxg", [FEAT, HID], BF, kind="ExternalInput")
    wh8_ext = nc.dram_tensor("wh8", [HID, G4], F8, kind="ExternalInput")
    wmu_ext = nc.dram_tensor("wmu", [HID, FEAT], BF, kind="ExternalInput")
    wsp8_ext = nc.dram_tensor("wsp8", [HID, FEAT], F8, kind="ExternalInput")
    we_ext = nc.dram_tensor("we", [SCENE, FEAT], BF, kind="ExternalInput")
    bias_ext = nc.dram_tensor("bias", [P, 38], F32, kind="ExternalInput")
    # outputs in bf16 (host upcasts): halves store DMA; the fed-back z is
    # identical to the bf16 x_next the LSTM consumed anyway
    oz_ext = nc.dram_tensor("oz", [NS, FEAT, b_loc], BF, kind="ExternalOutput")
    omu_ext = nc.dram_tensor("omu", [NS, FEAT, b_loc], BF, kind="ExternalOutput")
    osg_ext = nc.dram_tensor("osg", [NS, FEAT, b_loc], BF, kind="ExternalOutput")

    with tile.TileContext(nc) as tc:
        with (
            tc.tile_pool(name="wp", bufs=1) as wp,
            tc.tile_pool(name="work", bufs=2) as dp,
            tc.tile_pool(name="psum", bufs=4, space="PSUM") as pp,
        ):
            # DMA order matters: the encoder and slot 0 need only we/s/wx/bias,
            # so load those before the big (4MB) W_hh to let PE start early.
            # s for BOTH passes is loaded upfront: 8KB/partition buys zero
            # mid-kernel dependency at the pass boundary. The first k-tile of
            # pass-0's s goes absolutely first (it alone gates PE start).
            s_sb = wp.tile([P, KX, b_loc], BF, tag="s", name="s_sb")
            s_re = s_ext.rearrange("(po p) b -> p po b", p=P)
            nc.sync.dma_start(s_sb[:, 0:1, 0:w], s_re[:, 0:1, 0:w])
            we_sb = wp.tile([P, KX, FEAT], BF, tag="we", name="we_sb")
            nc.sync.dma_start(we_sb[:], we_ext.rearrange("(ko p) m -> p ko m", p=P))
            bias_sb = wp.tile([P, 38], F32, tag="bias", name="bias_sb")
            nc.sync.dma_start(bias_sb[:], bias_ext[:])
            nc.sync.dma_start(s_sb[:, 1:2, 0:w], s_re[:, 1:2, 0:w])
            wx8_sb = wp.tile([P, KX, G4], F8, tag="wx8", name="wx8_sb")
            nc.sync.dma_start(wx8_sb[:], wx8_ext.rearrange("(ko p) m -> p ko m", p=P))
            wxg_sb = wp.tile([P, KX, HID], BF, tag="wxg", name="wxg_sb")
            nc.sync.dma_start(wxg_sb[:], wxg_ext.rearrange("(ko p) m -> p ko m", p=P))
            nc.sync.dma_start(s_sb[:, :, w:b_loc], s_re[:, :, w:b_loc])
            wmu_sb = wp.tile([P, KH, FEAT], BF, tag="wmu", name="wmu_sb")
            nc.sync.dma_start(wmu_sb[:], wmu_ext.rearrange("(ko p) m -> p ko m", p=P))
            wsp8_sb = wp.tile([P, KH, FEAT], F8, tag="wsp8", name="wsp8_sb")
            nc.sync.dma_start(
                wsp8_sb[:], wsp8_ext.rearrange("(ko p) m -> p ko m", p=P)
            )
            wh8_sb = wp.tile([P, KH, G4], F8, tag="wh8", name="wh8_sb")
            nc.sync.dma_start(wh8_sb[:], wh8_ext.rearrange("(ko p) m -> p ko m", p=P))

            # x/x8 as manually alternated persistent buffers: pool-tag cycling
            # across the encoder/slot scopes makes tile_validation fall back
            # to a conservative min-join that stalls the PE sequencer on the
            # full DVE drain at pass boundaries. The encoder gets dedicated
            # buffers so the NEXT pass's encoder can be emitted mid-pass.
            xb = [
                wp.tile([P, KX, w], BF, tag=f"x{i}", name=f"x{i}_sb")
                for i in range(2)
            ]
            x8b = [
                wp.tile([P, KX, w], F8, tag=f"x8{i}", name=f"x8{i}_sb")
                for i in range(2)
            ]
            xe = wp.tile([P, KX, w], BF, tag="xe", name="xe_sb")
            x8e = wp.tile([P, KX, w], F8, tag="x8e", name="x8e_sb")

            def emit_encoder(pi):
                # x0 = gelu(We @ S_loc.T + be), feature-major, into xe/x8e
                ec0 = pi * w
                for fi in range(PO):
                    ps = pp.tile([P, w], F32, tag="ps", name="ps_enc")
                    for k in range(KX):
                        for ci, (c0, cw) in enumerate(chunks):
                            nc.tensor.matmul(
                                ps[:, c0 : c0 + cw],
                                we_sb[:, k, fi * P : (fi + 1) * P],
                                s_sb[:, k, ec0 + c0 : ec0 + c0 + cw],
                                start=(k == 0),
                                stop=(k == KX - 1),
                            )
                    nc.scalar.activation(
                        xe[:, fi],
                        ps[:],
                        AF.Gelu,
                        bias=bias_sb[:, 32 + fi : 33 + fi],
                    )
                for fi in range(PO):
                    nc.vector.tensor_scalar_mul(x8e[:, fi], xe[:, fi], SA)

            for p_i in range(n_pass):
                col0 = p_i * w
                c_sb = dp.tile([P, KH, w], F16, tag="c", bufs=1, name="c_sb")

                if p_i == 0:
                    emit_encoder(0)
                x_cur = xe
                x8_cur = x8e

                h_prev = None
                h8_prev = None
                for t in range(n_slots):
                    eps_sb = dp.tile([P, PO, w], BF, tag="eps", bufs=2, name="eps_sb")
                    nc.sync.dma_start(
                        eps_sb[:],
                        eps_ext[t].rearrange("(po p) b -> p po b", p=P)[
                            :, :, col0 : col0 + w
                        ],
                    )
                    h_new = dp.tile([P, KH, w], BF, tag="h", bufs=2, name="h_sb")
                    h8_new = dp.tile([P, KH, w], F8, tag="h8", bufs=2, name="h8_sb")
                    # (r_idx, o_gate_tile) whose tanh(c)/h-multiply is
                    # deferred until after the NEXT r-block's gate evictions,
                    # so ScalarE's eviction stream never stalls on the DVE
                    # c-update (that stall starves PSUM recycling and PE).
                    pend = None

                    def flush_pend():
                        nonlocal pend
                        if pend is None:
                            return
                        rp, gop = pend
                        th = dp.tile([P, w], BF, tag="th", bufs=2, name="th_sb")
                        nc.scalar.activation(th[:], c_sb[:, rp], AF.Tanh)
                        nc.vector.tensor_mul(h_new[:, rp], gop[:], th[:])
                        # h8 = (o*16)*tanh(c) in one DVE op (fp8 out)
                        nc.vector.scalar_tensor_tensor(
                            h8_new[:, rp], gop[:], SA, th[:], MUL, MUL
                        )
                        pend = None

                    def emit_h_part(ps, m):
                        for kk in range(KH // 2):
                            for ci, (c0, cw) in enumerate(chunks):
                                nc.tensor.matmul(
                                    ps[:, c0 : c0 + cw],
                                    wh8_sb[:, 2 * kk : 2 * kk + 2, m * P : (m + 1) * P],
                                    h8_prev[:, 2 * kk : 2 * kk + 2, c0 : c0 + cw],
                                    start=(kk == 0),
                                    stop=False,
                                    perf_mode=DR,
                                )

                    def emit_x_part(ps, g, r, m, first):
                        if g == 2:
                            # tanh gate: bf16 x-path (W pre-scaled 4096)
                            for k in range(KX):
                                for ci, (c0, cw) in enumerate(chunks):
                                    nc.tensor.matmul(
                                        ps[:, c0 : c0 + cw],
                                        wxg_sb[:, k, r * P : (r + 1) * P],
                                        x_cur[:, k, c0 : c0 + cw],
                                        start=(first and k == 0),
                                        stop=(k == KX - 1),
                                    )
                        else:
                            for ci, (c0, cw) in enumerate(chunks):
                                nc.tensor.matmul(
                                    ps[:, c0 : c0 + cw],
                                    wx8_sb[:, 0:2, m * P : (m + 1) * P],
                                    x8_cur[:, 0:2, c0 : c0 + cw],
                                    start=first,
                                    stop=True,
                                    perf_mode=DR,
                                )

                    for r in range(KH):
                        gts = {}
                        # slot 0: c=0, so the forget gate is never used
                        gate_ids = (0, 2, 3) if h_prev is None else (0, 1, 2, 3)
                        pss = {}
                        if h_prev is not None and r == 0:
                            # first r-block of a slot: issue ALL h-side chains
                            # before any x-part, so the x8/x feedback chain
                            # from the previous slot's heads hides under ~7us
                            # of h-matmul work.
                            for g in gate_ids:
                                pss[g] = pp.tile([P, w], F32, tag="ps", name="ps_g")
                                emit_h_part(pss[g], g * KH + r)
                        for g in gate_ids:
                            m = g * KH + r
                            if g in pss:
                                ps = pss[g]
                                emit_x_part(ps, g, r, m, False)
                            else:
                                ps = pp.tile([P, w], F32, tag="ps", name="ps_g")
                                first = True
                                if h_prev is not None:
                                    emit_h_part(ps, m)
                                    first = False
                                emit_x_part(ps, g, r, m, first)
                            # o-gate is double-buffered: its consumer (the
                            # deferred h-multiply) runs one r-block late
                            gt = dp.tile(
                                [P, w],
                                BF,
                                tag=f"g{g}",
                                bufs=2 if g == 3 else 1,
                                name=f"g{g}_sb",
                            )
                            func = AF.Tanh if g == 2 else AF.Sigmoid
                            nc.scalar.activation(
                                gt[:],
                                ps[:],
                                func,
                                bias=bias_sb[:, m : m + 1],
                                scale=INV_G,
                            )
                            gts[g] = gt
                            # flush the previous r-block's deferred tanh/h
                            # chain right after the FIRST eviction: its th
                            # lands one eviction deep in the ScalarE queue
                            # instead of four, shortening the h8 straggler
                            # chain at slot boundaries.
                            if g == gate_ids[0]:
                                flush_pend()
                        gi, gf, gg, go = (gts.get(g) for g in range(4))
                        if h_prev is not None:
                            t1 = dp.tile([P, w], BF, tag="t1", bufs=1, name="t1_sb")
                            nc.vector.tensor_mul(t1[:], gi[:], gg[:])
                            t2 = dp.tile([P, w], F32, tag="t2", bufs=2, name="t2_sb")
                            nc.vector.tensor_mul(t2[:], gf[:], c_sb[:, r])
                            nc.vector.tensor_add(c_sb[:, r], t1[:], t2[:])
                        else:
                            nc.vector.tensor_mul(c_sb[:, r], gi[:], gg[:])
                        pend = (r, go)
                    flush_pend()

                    # heads: mu = Wmu.T @ h (bf16); softplus_pre = Wsp.T @ h8
                    # (fp8 DR). The sigma half runs first: its ACT chain
                    # (Exp -> Ln -> ze) is the long pole toward the next
                    # slot's x, and the Ln/ze ops overlap the mu matmuls.
                    mu_sb = dp.tile([P, PO, w], BF, tag="mu", bufs=1, name="mu_sb")
                    sg_sb = dp.tile([P, PO, w], BF, tag="sg", bufs=1, name="sg_sb")
                    x_next = xb[t % 2]
                    x8_next = x8b[t % 2]
                    last_slot = p_i == n_pass - 1 and t == n_slots - 1
                    def emit_sp(po, ps_sp, kks, start, stop):
                        for kk in kks:
                            for ci, (c0, cw) in enumerate(chunks):
                                nc.tensor.matmul(
                                    ps_sp[:, c0 : c0 + cw],
                                    wsp8_sb[
                                        :, 2 * kk : 2 * kk + 2, po * P : (po + 1) * P
                                    ],
                                    h8_new[:, 2 * kk : 2 * kk + 2, c0 : c0 + cw],
                                    start=(start and kk == kks[0]),
                                    stop=(stop and kk == kks[-1]),
                                    perf_mode=DR,
                                )

                    def emit_mu(po, ps_mu, ks, start, stop):
                        for k in ks:
                            for ci, (c0, cw) in enumerate(chunks):
                                nc.tensor.matmul(
                                    ps_mu[:, c0 : c0 + cw],
                                    wmu_sb[:, k, po * P : (po + 1) * P],
                                    h_new[:, k, c0 : c0 + cw],
                                    start=(start and k == ks[0]),
                                    stop=(stop and k == ks[-1]),
                                )

                    def sp_acts(po, ps_sp):
                        # softplus(u) = ln(1 + exp(u)); this toolchain has no
                        # softplus ACT table, but exp and ln share one.
                        nc.scalar.activation(
                            sg_sb[:, po],
                            ps_sp[:],
                            AF.Exp,
                            bias=bias_sb[:, 36 + po : 37 + po],
                            scale=INV_SP,
                        )
                        nc.scalar.activation(
                            sg_sb[:, po], sg_sb[:, po], AF.Ln, bias=1.0
                        )
                        zt = dp.tile([P, w], F32, tag="t2", bufs=2, name="ze_sb")
                        nc.vector.tensor_mul(zt[:], sg_sb[:, po], eps_sb[:, po])
                        return zt

                    def mu_acts(po, ps_mu, zt):
                        nc.scalar.activation(
                            mu_sb[:, po],
                            ps_mu[:],
                            AF.Identity,
                            bias=bias_sb[:, 34 + po : 35 + po],
                        )
                        # z = mu + sigma*eps, written straight into the bf16
                        # x_next tile: it IS both the next slot's input and
                        # the z output
                        nc.vector.tensor_add(x_next[:, po], zt[:], mu_sb[:, po])
                        if not last_slot:
                            nc.vector.tensor_scalar_mul(
                                x8_next[:, po], x_next[:, po], SA
                            )
                        nc.sync.dma_start(
                            oz_ext[t].rearrange("(po p) b -> p po b", p=P)[
                                :, po : po + 1, col0 : col0 + w
                            ],
                            x_next[:, po : po + 1],
                        )
                        nc.sync.dma_start(
                            omu_ext[t].rearrange("(po p) b -> p po b", p=P)[
                                :, po : po + 1, col0 : col0 + w
                            ],
                            mu_sb[:, po : po + 1],
                        )

                    def sg_dma(po):
                        nc.sync.dma_start(
                            osg_ext[t].rearrange("(po p) b -> p po b", p=P)[
                                :, po : po + 1, col0 : col0 + w
                            ],
                            sg_sb[:, po : po + 1],
                        )

                    if last_slot:
                        # tail-optimized: both sigma chains first (their ACT
                        # chain is the long pole), mu after — nothing feeds
                        # back, only DMAs remain.
                        zts = []
                        for po in range(PO):
                            ps_sp = pp.tile([P, w], F32, tag="ps", name="ps_h")
                            emit_sp(po, ps_sp, [0, 1, 2, 3], True, True)
                            zts.append(sp_acts(po, ps_sp))
                            sg_dma(po)
                        for po in range(PO):
                            ps_mu = pp.tile([P, w], F32, tag="ps", name="ps_h")
                            emit_mu(po, ps_mu, list(range(KH)), True, True)
                            mu_acts(po, ps_mu, zts[po])
                        x_cur = x_next
                        x8_cur = x8_next
                        h_prev = h_new
                        h8_prev = h8_new
                        continue

                    # per po half: sp kk0..2, mu k0..5, sp kk3, mu k6..7 —
                    # the r6/r7 straggler chain (last gate evict -> tanh ->
                    # h/h8) hides under the r0..r5-only matmuls, and sigma's
                    # ACT chain (Exp -> Ln -> ze) still overlaps mu work.
                    for po in range(PO):
                        ps_sp = pp.tile([P, w], F32, tag="ps", name="ps_h")
                        emit_sp(po, ps_sp, [0, 1, 2], True, False)
                        ps_mu = pp.tile([P, w], F32, tag="ps", name="ps_h")
                        emit_mu(po, ps_mu, [0, 1, 2, 3, 4, 5], True, False)
                        emit_sp(po, ps_sp, [3], False, True)
                        zt = sp_acts(po, ps_sp)
                        emit_mu(po, ps_mu, [6, 7], False, True)
                        sg_dma(po)
                        mu_acts(po, ps_mu, zt)
                    x_cur = x_next
                    x8_cur = x8_next
                    h_prev = h_new
                    h8_prev = h8_new
                    if t == n_slots - 2 and p_i + 1 < n_pass:
                        # emit the NEXT pass's encoder here: its gelu/table
                        # swaps ride the ACT slack of slots 6-7 instead of
                        # sitting on the pass-boundary critical path.
                        emit_encoder(p_i + 1)
    return nc


_NC_CACHE = {}


def _get_nc(b_loc, w, n_slots):
    key = (b_loc, w, n_slots)
    if key not in _NC_CACHE:
        _NC_CACHE[key] = build(b_loc, w, n_slots)
    return _NC_CACHE[key]


def _prep_shared(We, be, W_ih, W_hh, b_ih, b_hh, W_mu, b_mu, W_sp, b_sp):
    f32 = np.float32
    wxT = np.ascontiguousarray(np.asarray(W_ih, f32).T)  # [FEAT, G4]
    wx8 = (wxT * SW).astype(F8E4)
    wxg = np.ascontiguousarray(wxT[:, 2 * HID : 3 * HID] * (SW * SA)).astype(BF16)
    wh8 = (np.ascontiguousarray(np.asarray(W_hh, f32).T) * SW).astype(F8E4)
    wmu = np.ascontiguousarray(np.asarray(W_mu, f32).T).astype(BF16)
    wsp8 = (np.ascontiguousarray(np.asarray(W_sp, f32).T) * SSP).astype(F8E4)
    we = np.ascontiguousarray(np.asarray(We, f32).T).astype(BF16)
    bg = (np.asarray(b_ih, f32) + np.asarray(b_hh, f32)).reshape(MT, P).T
    beT = np.asarray(be, f32).reshape(PO, P).T
    bmuT = np.asarray(b_mu, f32).reshape(PO, P).T
    bspT = np.asarray(b_sp, f32).reshape(PO, P).T
    bias = np.ascontiguousarray(
        np.concatenate([bg, beT, bmuT, bspT], axis=1), dtype=f32
    )
    return {
        "wx8": wx8,
        "wxg": wxg,
        "wh8": wh8,
        "wmu": wmu,
        "wsp8": wsp8,
        "we": we,
        "bias": bias,
    }


def _prep_in_maps(S, eps, shared, n_cores=N_CORES, b_loc=B_LOC):
    f32 = np.float32
    S = np.asarray(S, f32)
    eps = np.asarray(eps, f32)
    in_maps = []
    for ci in range(n_cores):
        rows = slice(ci * b_loc, (ci + 1) * b_loc)
        s_t = np.ascontiguousarray(S[rows].T).astype(BF16)
        eps_t = np.ascontiguousarray(eps[:NS, rows, :].transpose(0, 2, 1)).astype(
            BF16
        )
        in_maps.append({"s": s_t, "eps": eps_t, **shared})
    return in_maps


def _run(inputs, trace=False):
    from concourse.bass_utils import run_bass_kernel_spmd

    num_slots = int(inputs.get("num_slots", NS))
    nc = _get_nc(B_LOC, PASS_W, NS)
    shared = _prep_shared(
        inputs["We"], inputs["be"], inputs["W_ih"], inputs["W_hh"],
        inputs["b_ih"], inputs["b_hh"], inputs["W_mu"], inputs["b_mu"],
        inputs["W_sp"], inputs["b_sp"],
    )
    in_maps = _prep_in_maps(inputs["S"], inputs["eps"], shared)
    res = run_bass_kernel_spmd(
        nc, in_maps, core_ids=list(range(N_CORES)), trace=trace
    )
    zs = np.empty((NS, B, FEAT), np.float32)
    mus = np.empty((NS, B, FEAT), np.float32)
    sgs = np.empty((NS, B, FEAT), np.float32)
    for ci in range(N_CORES):
        rows = slice(ci * B_LOC, (ci + 1) * B_LOC)
        zs[:, rows, :] = res.results[ci]["oz"].transpose(0, 2, 1)
        mus[:, rows, :] = res.results[ci]["omu"].transpose(0, 2, 1)
        sgs[:, rows, :] = res.results[ci]["osg"].transpose(0, 2, 1)
    return (zs[:num_slots], mus[:num_slots], sgs[:num_slots]), res.exec_time_ns


def kernel(**inputs):
    out, _ = _run(inputs, trace=False)
    return out


# revision 30
# speedup vs baseline: 1.0208x; 1.0161x over previous
"""Trainium2 Bass kernel for nn_AutoregressivePrior (8-slot LSTM prior).

Strategy: pure data-parallel over batch (16384 rows -> 2048 per NeuronCore),
weights replicated. Feature-major dataflow on chip: every activation lives as
[feature_partition, batch_free] so LSTM matmul chains never transpose.

Mixed-precision matmuls tuned to the PE roofline (fp8 DoubleRow = 2x bf16):
  - W_hh @ h (70%% of FLOPs): fp8 e4m3 DoubleRow, W scaled 2^8, h scaled 2^4.
  - W_ih @ x for the i/f/o gates: fp8 DoubleRow (same scales).
  - W_ih @ x for the g gate (tanh, error-sensitive): bf16, W pre-scaled 2^12
    so its PSUM matches the fp8 product scale; one 1/4096 on the way out.
  - softplus head: fp8 DoubleRow reusing the gates' h8 tiles; mu head: bf16
    (mu is a direct output, fp8 there costs ~4%% rel err).
PSUM accumulates fp32; gate nonlinearities fused with bias+descale on ScalarE
straight out of PSUM; cell state held in fp16 in SBUF.

Per core the 2048-row batch is processed in two sequential passes of 1024
columns so all state fits in SBUF.

Inputs arrive as full-size numpy arrays; outputs are returned full-size
(zs, mus, sigmas) each [num_slots, 16384, 256] fp32, matching the reference.
"""

import sys

if "/opt/trn_rl_repo" not in sys.path:
    sys.path.insert(0, "/opt/trn_rl_repo")

import numpy as np
import ml_dtypes

BF16 = ml_dtypes.bfloat16
F8E4 = ml_dtypes.float8_e4m3

B = 16384
N_CORES = 8
B_LOC = B // N_CORES  # 2048
SCENE = 256
FEAT = 256
HID = 1024
G4 = 4 * HID  # 4096
NS = 8
P = 128
PO = FEAT // P  # 2
KH = HID // P  # 8
KX = FEAT // P  # 2
MT = G4 // P  # 32

PASS_W = 1024  # batch columns per pass on chip

SW = 256.0  # fp8 weight scale
SA = 16.0  # fp8 activation scale
INV_G = 1.0 / (SW * SA)  # gate PSUM descale (both fp8 and scaled-bf16 paths)
SSP = 128.0  # fp8 softplus-head weight scale
INV_SP = 1.0 / (SSP * SA)

_PATCHED = False


def _patch_tile_drain():
    """walrus in this toolchain rejects >1 sync-wait on a single instruction;
    split excess waits onto standalone single-wait EventSemaphore instructions
    that run on the same engine immediately before the original instruction."""
    global _PATCHED
    if _PATCHED:
        return
    import bass_rust
    import concourse.tile as tile
    from concourse import mybir
    from concourse.vector_clock import ScopedClock

    MAXW = 1
    _orig_lower = tile.TileContext._lower_ordered_insts

    def _lower_split_waits(self, ordered):
        nc = self.nc
        for bbn, insts in ordered.items():
            out = []
            for inst in insts:
                si = getattr(inst, "sync_info", None)
                if si is not None:
                    waits = list(si.on_wait)
                    if len(waits) > MAXW:
                        imm = [w for w in waits if w.wait_mode == "sem-ge-imm"]
                        other = [w for w in waits if w.wait_mode != "sem-ge-imm"]
                        assert len(other) <= MAXW, (inst.name, waits)
                        keep_n = MAXW - len(other)
                        if keep_n > 0:
                            move = imm[: len(imm) - keep_n]
                            keep = imm[len(imm) - keep_n :]
                        else:
                            move = imm
                            keep = []
                        for wt in move:
                            wi = mybir.InstEventSemaphore(
                                name=nc.get_next_instruction_name(),
                                ins=[],
                                outs=[],
                                engine=inst.engine,
                            )
                            wi.sync_info = bass_rust.SyncInfo(
                                on_wait=[wt], on_update=[]
                            )
                            out.append(wi)
                        si.on_wait = other + keep
                out.append(inst)
            insts[:] = out
        return _orig_lower(self, ordered)

    tile.TileContext._lower_ordered_insts = _lower_split_waits

    def _drain_and_barrier(self, tick_clock, wait_clock):
        nc = self.nc
        drain_inst = nc.sync.drain()
        wait_clock.add_sem_waits(
            drain_inst.ins, ScopedClock({None: tick_clock.global_clock})
        )
        si = drain_inst.ins.sync_info
        if si is not None and len(si.on_wait) > 1:
            waits = list(si.on_wait)
            si.on_wait = waits[:1]
            name2handle = {h.name: h for h in self.sems.allocated().values()}
            for w in waits[1:]:
                assert w.wait_mode == "sem-ge-imm", w
                nc.sync.wait_ge(name2handle[w.ant_name], w.wait_value)
        nc.all_engine_barrier()
        popped = nc._tile_sem_poison_stack.pop()
        assert popped is self._sem_poison
        nc.clear_and_free_semaphores(list(self.sems.allocated().values()))
        nc.all_engine_barrier()

    tile.TileContext._drain_and_barrier = _drain_and_barrier
    _PATCHED = True


def build(b_loc=B_LOC, w=PASS_W, n_slots=NS, mm_n=512):
    _patch_tile_drain()
    import concourse.bass as bass
    import concourse.tile as tile
    from concourse import mybir

    F32 = mybir.dt.float32
    BF = mybir.dt.bfloat16
    F16 = mybir.dt.float16
    F8 = mybir.dt.float8e4
    AF = mybir.ActivationFunctionType
    DR = mybir.MatmulPerfMode.DoubleRow
    MUL = mybir.AluOpType.mult

    n_pass = b_loc // w
    assert n_pass * w == b_loc
    chunks = [(c, min(mm_n, w - c)) for c in range(0, w, mm_n)]

    nc = bass.Bass()
    s_ext = nc.dram_tensor("s", [SCENE, b_loc], BF, kind="ExternalInput")
    eps_ext = nc.dram_tensor("eps", [NS, FEAT, b_loc], BF, kind="ExternalInput")
    wx8_ext = nc.dram_tensor("wx8", [FEAT, G4], F8, kind="ExternalInput")
    wxg_ext = nc.dram_tensor("wxg", [FEAT, HID], BF, kind="ExternalInput")
    wh8_ext = nc.dram_tensor("wh8", [HID, G4], F8, kind="ExternalInput")
    wmu_ext = nc.dram_tensor("wmu", [HID, FEAT], BF, kind="ExternalInput")
    wsp8_ext = nc.dram_tensor("wsp8", [HID, FEAT], F8, kind="ExternalInput")
    we_ext = nc.dram_tensor("we", [SCENE, FEAT], BF, kind="ExternalInput")
    bias_ext = nc.dram_tensor("bias", [P, 38], F32, kind="ExternalInput")
    # outputs in bf16 (host upcasts): halves store DMA; the fed-back z is
    # identical to the bf16 x_next the LSTM consumed anyway
    oz_ext = nc.dram_tensor("oz", [NS, FEAT, b_loc], BF, kind="ExternalOutput")
    omu_ext = nc.dram_tensor("omu", [NS, FEAT, b_loc], BF, kind="ExternalOutput")
    osg_ext = nc.dram_tensor("osg", [NS, FEAT, b_loc], BF, kind="ExternalOutput")

    with tile.TileContext(nc) as tc:
        with (
            tc.tile_pool(name="wp", bufs=1) as wp,
            tc.tile_pool(name="work", bufs=2) as dp,
            tc.tile_pool(name="psum", bufs=4, space="PSUM") as pp,
        ):
            # DMA order matters: the encoder and slot 0 need only we/s/wx/bias,
            # so load those before the big (4MB) W_hh to let PE start early.
            # s for BOTH passes is loaded upfront: 8KB/partition buys zero
            # mid-kernel dependency at the pass boundary. The first k-tile of
            # pass-0's s goes absolutely first (it alone gates PE start).
            s_sb = wp.tile([P, KX, b_loc], BF, tag="s", name="s_sb")
            s_re = s_ext.rearrange("(po p) b -> p po b", p=P)
            nc.sync.dma_start(s_sb[:, 0:1, 0:w], s_re[:, 0:1, 0:w])
            we_sb = wp.tile([P, KX, FEAT], BF, tag="we", name="we_sb")
            nc.sync.dma_start(we_sb[:], we_ext.rearrange("(ko p) m -> p ko m", p=P))
            bias_sb = wp.tile([P, 38], F32, tag="bias", name="bias_sb")
            nc.sync.dma_start(bias_sb[:], bias_ext[:])
            nc.sync.dma_start(s_sb[:, 1:2, 0:w], s_re[:, 1:2, 0:w])
            wx8_sb = wp.tile([P, KX, G4], F8, tag="wx8", name="wx8_sb")
            nc.sync.dma_start(wx8_sb[:], wx8_ext.rearrange("(ko p) m -> p ko m", p=P))
            wxg_sb = wp.tile([P, KX, HID], BF, tag="wxg", name="wxg_sb")
            nc.sync.dma_start(wxg_sb[:], wxg_ext.rearrange("(ko p) m -> p ko m", p=P))
            nc.sync.dma_start(s_sb[:, :, w:b_loc], s_re[:, :, w:b_loc])
            wmu_sb = wp.tile([P, KH, FEAT], BF, tag="wmu", name="wmu_sb")
            nc.sync.dma_start(wmu_sb[:], wmu_ext.rearrange("(ko p) m -> p ko m", p=P))
            wsp8_sb = wp.tile([P, KH, FEAT], F8, tag="wsp8", name="wsp8_sb")
            nc.sync.dma_start(
                wsp8_sb[:], wsp8_ext.rearrange("(ko p) m -> p ko m", p=P)
            )
            wh8_sb = wp.tile([P, KH, G4], F8, tag="wh8", name="wh8_sb")
            nc.sync.dma_start(wh8_sb[:], wh8_ext.rearrange("(ko p) m -> p ko m", p=P))

            # x/x8 as manually alternated persistent buffers: pool-tag cycling
            # across the encoder/slot scopes makes tile_validation fall back
            # to a conservative min-join that stalls the PE sequencer on the
            # full DVE drain at pass boundaries. The encoder gets dedicated
            # buffers so the NEXT pass's encoder can be emitted mid-pass.
            xb = [
                wp.tile([P, KX, w], BF, tag=f"x{i}", name=f"x{i}_sb")
                for i in range(2)
            ]
            x8b = [
                wp.tile([P, KX, w], F8, tag=f"x8{i}", name=f"x8{i}_sb")
                for i in range(2)
            ]
            xe = wp.tile([P, KX, w], BF, tag="xe", name="xe_sb")
            x8e = wp.tile([P, KX, w], F8, tag="x8e", name="x8e_sb")

            def emit_encoder(pi):
                # x0 = gelu(We @ S_loc.T + be), feature-major, into xe/x8e
                ec0 = pi * w
                for fi in range(PO):
                    ps = pp.tile([P, w], F32, tag="ps", name="ps_enc")
                    for k in range(KX):
                        for ci, (c0, cw) in enumerate(chunks):
                            nc.tensor.matmul(
                                ps[:, c0 : c0 + cw],
                                we_sb[:, k, fi * P : (fi + 1) * P],
                                s_sb[:, k, ec0 + c0 : ec0 + c0 + cw],
                                start=(k == 0),
                                stop=(k == KX - 1),
                            )
                    nc.scalar.activation(
                        xe[:, fi],
                        ps[:],
                        AF.Gelu,
                        bias=bias_sb[:, 32 + fi : 33 + fi],
                    )
                for fi in range(PO):
                    nc.vector.tensor_scalar_mul(x8e[:, fi], xe[:, fi], SA)

            for p_i in range(n_pass):
                col0 = p_i * w
                c_sb = dp.tile([P, KH, w], F16, tag="c", bufs=1, name="c_sb")

                if p_i == 0:
                    emit_encoder(0)
                x_cur = xe
                x8_cur = x8e

                h_prev = None
                h8_prev = None
                for t in range(n_slots):
                    eps_sb = dp.tile([P, PO, w], BF, tag="eps", bufs=2, name="eps_sb")
                    nc.sync.dma_start(
                        eps_sb[:],
                        eps_ext[t].rearrange("(po p) b -> p po b", p=P)[
                            :, :, col0 : col0 + w
                        ],
                    )
                    h_new = dp.tile([P, KH, w], BF, tag="h", bufs=2, name="h_sb")
                    h8_new = dp.tile([P, KH, w], F8, tag="h8", bufs=2, name="h8_sb")
                    # (r_idx, o_gate_tile) whose tanh(c)/h-multiply is
                    # deferred until after the NEXT r-block's gate evictions,
                    # so ScalarE's eviction stream never stalls on the DVE
                    # c-update (that stall starves PSUM recycling and PE).
                    pend = None

                    def flush_pend():
                        nonlocal pend
                        if pend is None:
                            return
                        rp, gop = pend
                        th = dp.tile([P, w], BF, tag="th", bufs=2, name="th_sb")
                        nc.scalar.activation(th[:], c_sb[:, rp], AF.Tanh)
                        nc.vector.tensor_mul(h_new[:, rp], gop[:], th[:])
                        # h8 = (o*16)*tanh(c) in one DVE op (fp8 out)
                        nc.vector.scalar_tensor_tensor(
                            h8_new[:, rp], gop[:], SA, th[:], MUL, MUL
                        )
                        pend = None

                    def emit_h_part(ps, m):
                        for kk in range(KH // 2):
                            for ci, (c0, cw) in enumerate(chunks):
                                nc.tensor.matmul(
                                    ps[:, c0 : c0 + cw],
                                    wh8_sb[:, 2 * kk : 2 * kk + 2, m * P : (m + 1) * P],
                                    h8_prev[:, 2 * kk : 2 * kk + 2, c0 : c0 + cw],
                                    start=(kk == 0),
                                    stop=False,
                                    perf_mode=DR,
                                )

                    def emit_x_part(ps, g, r, m, first):
                        if g == 2:
                            # tanh gate: bf16 x-path (W pre-scaled 4096)
                            for k in range(KX):
                                for ci, (c0, cw) in enumerate(chunks):
                                    nc.tensor.matmul(
                                        ps[:, c0 : c0 + cw],
                                        wxg_sb[:, k, r * P : (r + 1) * P],
                                        x_cur[:, k, c0 : c0 + cw],
                                        start=(first and k == 0),
                                        stop=(k == KX - 1),
                                    )
                        else:
                            for ci, (c0, cw) in enumerate(chunks):
                                nc.tensor.matmul(
                                    ps[:, c0 : c0 + cw],
                                    wx8_sb[:, 0:2, m * P : (m + 1) * P],
                                    x8_cur[:, 0:2, c0 : c0 + cw],
                                    start=first,
                                    stop=True,
                                    perf_mode=DR,
                                )

                    for r in range(KH):
                        gts = {}
                        # slot 0: c=0, so the forget gate is never used
                        gate_ids = (0, 2, 3) if h_prev is None else (0, 1, 2, 3)
                        pss = {}
                        if h_prev is not None and r == 0:
                            # first r-block of a slot: issue ALL h-side chains
                            # before any x-part, so the x8/x feedback chain
                            # from the previous slot's heads hides under ~7us
                            # of h-matmul work.
                            for g in gate_ids:
                                pss[g] = pp.tile([P, w], F32, tag="ps", name="ps_g")
                                emit_h_part(pss[g], g * KH + r)
                        for g in gate_ids:
                            m = g * KH + r
                            if g in pss:
                                ps = pss[g]
                                emit_x_part(ps, g, r, m, False)
                            else:
                                ps = pp.tile([P, w], F32, tag="ps", name="ps_g")
                                first = True
                                if h_prev is not None:
                                    emit_h_part(ps, m)
                                    first = False
                                emit_x_part(ps, g, r, m, first)
                            # o-gate is double-buffered: its consumer (the
                            # deferred h-multiply) runs one r-block late
                            gt = dp.tile(
                                [P, w],
                                BF,
                                tag=f"g{g}",
                                bufs=2 if g == 3 else 1,
                                name=f"g{g}_sb",
                            )
                            func = AF.Tanh if g == 2 else AF.Sigmoid
                            nc.scalar.activation(
                                gt[:],
                                ps[:],
                                func,
                                bias=bias_sb[:, m : m + 1],
                                scale=INV_G,
                            )
                            gts[g] = gt
                            # flush the previous r-block's deferred tanh/h
                            # chain right after the FIRST eviction: its th
                            # lands one eviction deep in the ScalarE queue
                            # instead of four, shortening the h8 straggler
                            # chain at slot boundaries.
                            if g == gate_ids[0]:
                                flush_pend()
                        gi, gf, gg, go = (gts.get(g) for g in range(4))
                        if h_prev is not None:
                            t1 = dp.tile([P, w], BF, tag="t1", bufs=1, name="t1_sb")
                            nc.vector.tensor_mul(t1[:], gi[:], gg[:])
                            t2 = dp.tile([P, w], F32, tag="t2", bufs=2, name="t2_sb")
                            nc.vector.tensor_mul(t2[:], gf[:], c_sb[:, r])
                            nc.vector.tensor_add(c_sb[:, r], t1[:], t2[:])
                        else:
                            nc.vector.tensor_mul(c_sb[:, r], gi[:], gg[:])
                        pend = (r, go)
                    flush_pend()

                    # heads: mu = Wmu.T @ h (bf16); softplus_pre = Wsp.T @ h8
                    # (fp8 DR). The sigma half runs first: its ACT chain
                    # (Exp -> Ln -> ze) is the long pole toward the next
                    # slot's x, and the Ln/ze ops overlap the mu matmuls.
                    mu_sb = dp.tile([P, PO, w], BF, tag="mu", bufs=1, name="mu_sb")
                    sg_sb = dp.tile([P, PO, w], BF, tag="sg", bufs=1, name="sg_sb")
                    x_next = xb[t % 2]
                    x8_next = x8b[t % 2]
                    last_slot = p_i == n_pass - 1 and t == n_slots - 1
                    def emit_sp(po, ps_sp, kks, start, stop):
                        for kk in kks:
                            for ci, (c0, cw) in enumerate(chunks):
                                nc.tensor.matmul(
                                    ps_sp[:, c0 : c0 + cw],
                                    wsp8_sb[
                                        :, 2 * kk : 2 * kk + 2, po * P : (po + 1) * P
                                    ],
                                    h8_new[:, 2 * kk : 2 * kk + 2, c0 : c0 + cw],
                                    start=(start and kk == kks[0]),
                                    stop=(stop and kk == kks[-1]),
                                    perf_mode=DR,
                                )

                    def emit_mu(po, ps_mu, ks, start, stop):
                        for k in ks:
                            for ci, (c0, cw) in enumerate(chunks):
                                nc.tensor.matmul(
                                    ps_mu[:, c0 : c0 + cw],
                                    wmu_sb[:, k, po * P : (po + 1) * P],
                                    h_new[:, k, c0 : c0 + cw],
                                    start=(start and k == ks[0]),
                                    stop=(stop and k == ks[-1]),
                                )

                    def sp_acts(po, ps_sp):
                        # softplus(u) = ln(1 + exp(u)); this toolchain has no
                        # softplus ACT table, but exp and ln share one.
                        nc.scalar.activation(
                            sg_sb[:, po],
                            ps_sp[:],
                            AF.Exp,
                            bias=bias_sb[:, 36 + po : 37 + po],
                            scale=INV_SP,
                        )
                        nc.scalar.activation(
                            sg_sb[:, po], sg_sb[:, po], AF.Ln, bias=1.0
                        )
                        zt = dp.tile([P, w], BF, tag="ze", bufs=2, name="ze_sb")
                        nc.vector.tensor_mul(zt[:], sg_sb[:, po], eps_sb[:, po])
                        return zt

                    def mu_acts(po, ps_mu, zt):
                        nc.scalar.activation(
                            mu_sb[:, po],
                            ps_mu[:],
                            AF.Identity,
                            bias=bias_sb[:, 34 + po : 35 + po],
                        )
                        # z = mu + sigma*eps, written straight into the bf16
                        # x_next tile: it IS both the next slot's input and
                        # the z output
                        nc.vector.tensor_add(x_next[:, po], zt[:], mu_sb[:, po])
                        if t < n_slots - 1:
                            # slot 7's x8 is never consumed: the next pass's
                            # slot 0 reads the hoisted encoder's x8e
                            nc.vector.tensor_scalar_mul(
                                x8_next[:, po], x_next[:, po], SA
                            )
                        nc.sync.dma_start(
                            oz_ext[t].rearrange("(po p) b -> p po b", p=P)[
                                :, po : po + 1, col0 : col0 + w
                            ],
                            x_next[:, po : po + 1],
                        )
                        nc.sync.dma_start(
                            omu_ext[t].rearrange("(po p) b -> p po b", p=P)[
                                :, po : po + 1, col0 : col0 + w
                            ],
                            mu_sb[:, po : po + 1],
                        )

                    def sg_dma(po):
                        nc.sync.dma_start(
                            osg_ext[t].rearrange("(po p) b -> p po b", p=P)[
                                :, po : po + 1, col0 : col0 + w
                            ],
                            sg_sb[:, po : po + 1],
                        )

                    if last_slot:
                        # tail-optimized: both sigma chains first (their ACT
                        # chain is the long pole), mu after — nothing feeds
                        # back, only DMAs remain.
                        zts = []
                        for po in range(PO):
                            ps_sp = pp.tile([P, w], F32, tag="ps", name="ps_h")
                            emit_sp(po, ps_sp, [0, 1, 2, 3], True, True)
                            zts.append(sp_acts(po, ps_sp))
                            sg_dma(po)
                        for po in range(PO):
                            ps_mu = pp.tile([P, w], F32, tag="ps", name="ps_h")
                            emit_mu(po, ps_mu, list(range(KH)), True, True)
                            mu_acts(po, ps_mu, zts[po])
                        x_cur = x_next
                        x8_cur = x8_next
                        h_prev = h_new
                        h8_prev = h8_new
                        continue

                    # per po half: sp kk0..2, mu k0..5, sp kk3, mu k6..7 —
                    # the r6/r7 straggler chain (last gate evict -> tanh ->
                    # h/h8) hides under the r0..r5-only matmuls, and sigma's
                    # ACT chain (Exp -> Ln -> ze) still overlaps mu work.
                    for po in range(PO):
                        ps_sp = pp.tile([P, w], F32, tag="ps", name="ps_h")
                        emit_sp(po, ps_sp, [0, 1, 2], True, False)
                        ps_mu = pp.tile([P, w], F32, tag="ps", name="ps_h")
                        emit_mu(po, ps_mu, [0, 1, 2, 3, 4, 5], True, False)
                        emit_sp(po, ps_sp, [3], False, True)
                        zt = sp_acts(po, ps_sp)
                        emit_mu(po, ps_mu, [6, 7], False, True)
                        sg_dma(po)
                        mu_acts(po, ps_mu, zt)
                    x_cur = x_next
                    x8_cur = x8_next
                    h_prev = h_new
                    h8_prev = h8_new
                    if t == n_slots - 2 and p_i + 1 < n_pass:
                        # emit the NEXT pass's encoder here: its gelu/table
                        # swaps ride the ACT slack of slots 6-7 instead of
                        # sitting on the pass-boundary critical path.
                        emit_encoder(p_i + 1)
    return nc


_NC_CACHE = {}


def _get_nc(b_loc, w, n_slots):
    key = (b_loc, w, n_slots)
    if key not in _NC_CACHE:
        _NC_CACHE[key] = build(b_loc, w, n_slots)
    return _NC_CACHE[key]


def _prep_shared(We, be, W_ih, W_hh, b_ih, b_hh, W_mu, b_mu, W_sp, b_sp):
    f32 = np.float32
    wxT = np.ascontiguousarray(np.asarray(W_ih, f32).T)  # [FEAT, G4]
    wx8 = (wxT * SW).astype(F8E4)
    wxg = np.ascontiguousarray(wxT[:, 2 * HID : 3 * HID] * (SW * SA)).astype(BF16)
    wh8 = (np.ascontiguousarray(np.asarray(W_hh, f32).T) * SW).astype(F8E4)
    wmu = np.ascontiguousarray(np.asarray(W_mu, f32).T).astype(BF16)
    wsp8 = (np.ascontiguousarray(np.asarray(W_sp, f32).T) * SSP).astype(F8E4)
    we = np.ascontiguousarray(np.asarray(We, f32).T).astype(BF16)
    bg = (np.asarray(b_ih, f32) + np.asarray(b_hh, f32)).reshape(MT, P).T
    beT = np.asarray(be, f32).reshape(PO, P).T
    bmuT = np.asarray(b_mu, f32).reshape(PO, P).T
    bspT = np.asarray(b_sp, f32).reshape(PO, P).T
    bias = np.ascontiguousarray(
        np.concatenate([bg, beT, bmuT, bspT], axis=1), dtype=f32
    )
    return {
        "wx8": wx8,
        "wxg": wxg,
        "wh8": wh8,
        "wmu": wmu,
        "wsp8": wsp8,
        "we": we,
        "bias": bias,
    }


def _prep_in_maps(S, eps, shared, n_cores=N_CORES, b_loc=B_LOC):
    f32 = np.float32
    S = np.asarray(S, f32)
    eps = np.asarray(eps, f32)
    in_maps = []
    for ci in range(n_cores):
        rows = slice(ci * b_loc, (ci + 1) * b_loc)
        s_t = np.ascontiguousarray(S[rows].T).astype(BF16)
        eps_t = np.ascontiguousarray(eps[:NS, rows, :].transpose(0, 2, 1)).astype(
            BF16
        )
        in_maps.append({"s": s_t, "eps": eps_t, **shared})
    return in_maps


def _run(inputs, trace=False):
    from concourse.bass_utils import run_bass_kernel_spmd

    num_slots = int(inputs.get("num_slots", NS))
    nc = _get_nc(B_LOC, PASS_W, NS)
    shared = _prep_shared(
        inputs["We"], inputs["be"], inputs["W_ih"], inputs["W_hh"],
        inputs["b_ih"], inputs["b_hh"], inputs["W_mu"], inputs["b_mu"],
        inputs["W_sp"], inputs["b_sp"],
    )
    in_maps = _prep_in_maps(inputs["S"], inputs["eps"], shared)
    res = run_bass_kernel_spmd(
        nc, in_maps, core_ids=list(range(N_CORES)), trace=trace
    )
    zs = np.empty((NS, B, FEAT), np.float32)
    mus = np.empty((NS, B, FEAT), np.float32)
    sgs = np.empty((NS, B, FEAT), np.float32)
    for ci in range(N_CORES):
        rows = slice(ci * B_LOC, (ci + 1) * B_LOC)
        zs[:, rows, :] = res.results[ci]["oz"].transpose(0, 2, 1)
        mus[:, rows, :] = res.results[ci]["omu"].transpose(0, 2, 1)
        sgs[:, rows, :] = res.results[ci]["osg"].transpose(0, 2, 1)
    return (zs[:num_slots], mus[:num_slots], sgs[:num_slots]), res.exec_time_ns


def kernel(**inputs):
    out, _ = _run(inputs, trace=False)
    return out
